# revision 1
# baseline (speedup 1.0000x reference)
"""BiLSTM-CRF tagger loss on 8 Trainium2 NeuronCores.

Sharding (SPMD, one program for all 8 cores):
  - 4 example-groups of 8; core g in 0..3 runs the FORWARD LSTM for group g,
    core g+4 runs the BACKWARD LSTM for the same group (its inputs are
    time-reversed on the host, so the device program is identical).
  - Each core: embedding'd inputs -> input GEMM -> 256-step LSTM scan
    (weights stationary on PE, batch streamed) -> partial emissions.
  - Pairwise AllGather {g, g+4} exchanges partial emissions; each core forms
    full emissions (partner slab time-reversed via negative-step AP) and runs
    the CRF for all 8 group examples redundantly (keeps the program SPMD).
  - CRF denominator runs in the linear domain: aT' = (E.T @ aT) * exp(em_t)
    with E = exp(trans), renormalized every 8 steps. Numerator is one-hot
    dot products against host-precomputed index tensors.
  - Host: gathers per-group llh vectors from the forward cores, returns
    -mean(llh).

dtypes: matmul operands bf16 (validated: full-pipeline rel err ~1e-6 vs
fp32 reference); gate math / c state / emissions / CRF in fp32.
"""
import sys
import numpy as np

sys.path.insert(0, "/opt/trn_rl_repo")

import ml_dtypes

V, E, H, L, B, T = 32000, 300, 512, 17, 32, 256
NCORES = 8
BG = 8          # examples per group
KCH = 4         # H / 128
ECH = 3         # ceil(300+1 bias / 128)
EPAD = 384
RENORM = 8

bfl = ml_dtypes.bfloat16
f8l = ml_dtypes.float8_e4m3

# fp8 recurrent weights/state (validated on HW: rel err 1.3e-6 at T=256)
USE_FP8 = True
USE_DR = False       # fp8 DoubleRow recurrent matmuls
SCHED = 'base'       # 'chunk': kp-outer mm order + chunked epilogue overlap
USE_GEMM_DR = False  # fp8 DoubleRow input GEMM (fp8 xt/wih/xg)
TSPLIT = 4           # >1: time-split scan path (build_nc2)
WARM = 8             # warmup steps per window in the time-split path
SWI = True           # DoubleRowSwInterleave weight layout
XG_EPI = False       # fold xg into epilogue (skip ident matmul)
PQ4 = False          # pq_bufs=4 / ps_bufs=2
NCH = 2              # CRF chains per direction (bidi: 2*NCH strands)

_CACHE = {}


# ---------------------------------------------------------------- device ---
def build_nc(T_=T, reps=1, fp8=False, phases='all', nch=1, nfuse=1,
             pq_bufs=3, ps_bufs=3, gemm_act=False, sp_bufs=3,
             k_outer=False, use_dr=False, sched='base', gemm_dr=False):
    import concourse.bass as bass
    import concourse.bacc as bacc
    import concourse.mybir as mybir
    import concourse.tile as tile
    from concourse.bass import AP

    f32 = mybir.dt.float32
    bf16 = mybir.dt.bfloat16
    AF = mybir.ActivationFunctionType
    NTOK = BG * T_
    GCH = max(1, NTOK // 512)   # token chunks for GEMM
    CW = NTOK // GCH
    if sched == 'chunk':
        nfuse = 2

    nc = bacc.Bacc("TRN2", target_bir_lowering=False, debug=False)

    f8 = mybir.dt.float8e4
    x_dt = f8 if gemm_dr else bf16
    xt = nc.dram_tensor("xt", [128, ECH, NTOK], x_dt, kind="ExternalInput")
    wih = nc.dram_tensor("wih", [128, ECH, 16, 128], x_dt, kind="ExternalInput")
    whh_dt = f8 if fp8 else bf16
    whh = nc.dram_tensor("whh", [128, KCH, 16, 128], whh_dt, kind="ExternalInput")
    wcls = nc.dram_tensor("wcls", [128, KCH, L], bf16, kind="ExternalInput")
    bcls = nc.dram_tensor("bcls", [L, 1], f32, kind="ExternalInput")
    transm = nc.dram_tensor("transm", [L, L], f32, kind="ExternalInput")
    stv = nc.dram_tensor("stv", [L, 1], f32, kind="ExternalInput")
    etv = nc.dram_tensor("etv", [L, 1], f32, kind="ExternalInput")
    ohem = nc.dram_tensor("ohem", [L, NTOK], f32, kind="ExternalInput")
    ohtp = nc.dram_tensor("ohtp", [L, NTOK], f32, kind="ExternalInput")
    ohtt = nc.dram_tensor("ohtt", [L, NTOK], f32, kind="ExternalInput")
    ohse = nc.dram_tensor("ohse", [L, 2 * BG], f32, kind="ExternalInput")
    ident = nc.dram_tensor("ident", [128, 128], x_dt, kind="ExternalInput")

    llh_out = nc.dram_tensor("llh_out", [1, BG], f32, kind="ExternalOutput")
    DR = mybir.MatmulPerfMode.DoubleRow
    if gemm_dr:
        assert fp8

    cc_ins = [nc.dram_tensor(f"cc_in{r}", [L, NTOK], f32) for r in range(reps)]
    cc_outs = [nc.dram_tensor(f"cc_out{r}", [2, L, NTOK], f32) for r in range(reps)]

    with tile.TileContext(nc) as tc:
        with tc.tile_pool(name="const", bufs=1) as cp, \
             tc.tile_pool(name="state", bufs=sp_bufs) as sp, \
             tc.tile_pool(name="crf", bufs=3) as fp, \
             tc.tile_pool(name="pgemm", bufs=2, space="PSUM") as pg, \
             tc.tile_pool(name="pgates", bufs=pq_bufs, space="PSUM") as pq, \
             tc.tile_pool(name="psmall", bufs=ps_bufs, space="PSUM") as ps:

            # ---------------- loads ----------------
            xt_sb = cp.tile([128, ECH, NTOK], x_dt, name="xt_sb")
            nc.sync.dma_start(xt_sb[:], xt[:])
            wih_sb = cp.tile([128, ECH, 16, 128], x_dt, name="wih_sb")
            nc.sync.dma_start(wih_sb[:], wih[:])
            whh_sb = cp.tile([128, KCH, 16, 128], whh_dt, name="whh_sb")
            nc.sync.dma_start(whh_sb[:], whh[:])
            wcls_sb = cp.tile([128, KCH, L], bf16, name="wcls_sb")
            nc.sync.dma_start(wcls_sb[:], wcls[:])
            bcls_sb = cp.tile([L, 1], f32, name="bcls_sb")
            nc.sync.dma_start(bcls_sb[:], bcls[:])
            trans_sb = cp.tile([L, L], f32, name="trans_sb")
            nc.sync.dma_start(trans_sb[:], transm[:])
            stv_sb = cp.tile([L, 1], f32, name="stv_sb")
            nc.sync.dma_start(stv_sb[:], stv[:])
            etv_sb = cp.tile([L, 1], f32, name="etv_sb")
            nc.sync.dma_start(etv_sb[:], etv[:])
            ohem_sb = cp.tile([L, NTOK], f32, name="ohem_sb")
            nc.sync.dma_start(ohem_sb[:], ohem[:])
            ohtp_sb = cp.tile([L, NTOK], f32, name="ohtp_sb")
            nc.sync.dma_start(ohtp_sb[:], ohtp[:])
            ohtt_sb = cp.tile([L, NTOK], f32, name="ohtt_sb")
            nc.sync.dma_start(ohtt_sb[:], ohtt[:])
            ohse_sb = cp.tile([L, 2 * BG], f32, name="ohse_sb")
            nc.sync.dma_start(ohse_sb[:], ohse[:])
            ident_sb = cp.tile([128, 128], x_dt, name="ident_sb")
            nc.sync.dma_start(ident_sb[:], ident[:])

            xg_sb = cp.tile([128, 16, NTOK], x_dt if gemm_dr else bf16,
                            name="xg_sb")
            em_sb = cp.tile([L, NTOK], f32, name="em_sb")

            for rep in range(reps):
                # ---------------- phase 1: input GEMM ----------------
                for n in range(GCH):
                    cols = slice(n * CW, (n + 1) * CW)
                    for s in range(16):
                        gp = pg.tile([128, CW], f32, name="gp", tag="gemm")
                        if gemm_dr:
                            # k pair (0,1) via DoubleRow in 256-col halves
                            # (moving free dim cap 512), then k=2 single fp8
                            for hh in range(CW // 256):
                                hs = slice(hh * 256, (hh + 1) * 256)
                                hc = slice(n * CW + hh * 256,
                                           n * CW + (hh + 1) * 256)
                                nc.tensor.matmul(
                                    gp[:, hs], wih_sb[:, 0:2, s, :],
                                    xt_sb[:, 0:2, hc],
                                    start=True, stop=False, perf_mode=DR,
                                )
                                nc.tensor.matmul(
                                    gp[:, hs], wih_sb[:, 2, s, :],
                                    xt_sb[:, 2, hc],
                                    start=False, stop=True,
                                )
                        else:
                            for k in range(ECH):
                                nc.tensor.matmul(
                                    gp[:], wih_sb[:, k, s, :], xt_sb[:, k, cols],
                                    start=(k == 0), stop=(k == ECH - 1),
                                )
                        if gemm_act or s % 2 == 1:
                            nc.scalar.copy(xg_sb[:, s, cols], gp[:])
                        else:
                            nc.vector.tensor_copy(xg_sb[:, s, cols], gp[:])

                # ---------------- phase 2: LSTM scan ----------------
                h_all = cp.tile([128, KCH, NTOK], bf16, name="h_all")
                hq_dt = f8 if fp8 else bf16
                h_q = sp.tile([128, KCH * BG], hq_dt, name="h_q", tag="hq")
                nc.vector.memset(h_q[:], 0.0)
                c_prev = sp.tile([128, KCH * BG], f32, name="c_prev", tag="c")
                nc.vector.memset(c_prev[:], 0.0)
                xgv = xg_sb.rearrange("p (j q) n -> p j q n", q=4)

                for t in range(T_):
                    gp = pq.tile([128, 128], f32, name="gp_scan", tag="g")
                    gpv = gp.rearrange("p (s b) -> p s b", b=BG)
                    # xg folded in on the PE: psum = (scale*I).T @ xg_t
                    nc.tensor.matmul(
                        gp[:], ident_sb[:],
                        xg_sb[:, :, BG * t:BG * (t + 1)],
                        start=True, stop=False, skip_group_check=True,
                    )
                    if use_dr:
                        # fp8 DoubleRow: one matmul contracts a k-pair
                        # (256 h dims). kp-outer order: the kp=0 wave only
                        # needs h chunks 0-1, so next step's PE can start
                        # while this step's epilogue finishes chunks 2-3.
                        hqv = h_q.rearrange("p (j b) -> p j b", b=BG)
                        NKP = KCH // 2
                        if sched == 'chunk':
                            mm_order = ([(s, 0) for s in range(16)]
                                        + [(s, 1) for s in range(16)])
                        else:
                            mm_order = [(s, kp) for s in range(16)
                                        for kp in range(NKP)]
                        for s, kp in mm_order:
                            nc.tensor.matmul(
                                gpv[:, s, :],
                                whh_sb[:, 2 * kp:2 * kp + 2, s, :],
                                hqv[:, 2 * kp:2 * kp + 2, :],
                                start=False, stop=(kp == NKP - 1),
                                perf_mode=DR, skip_group_check=True,
                            )
                    else:
                        mm_order = ([(s, k) for k in range(KCH) for s in range(16)]
                                    if k_outer else
                                    [(s, k) for s in range(16) for k in range(KCH)])
                        for s, k in mm_order:
                            nc.tensor.matmul(
                                gpv[:, s, :], whh_sb[:, k, s, :],
                                h_q[:, k * BG:(k + 1) * BG],
                                start=False, stop=(k == KCH - 1),
                                skip_group_check=True,
                            )
                    hq_new = sp.tile([128, KCH * BG], hq_dt, name="hq_new", tag="hq")
                    c_new = sp.tile([128, KCH * BG], f32, name="c_new", tag="c")
                    if phases == 'fake_epi':
                        nc.vector.tensor_copy(hq_new[:], gp[:, 0:KCH * BG])
                        nc.vector.tensor_copy(
                            h_all[:, :, BG * t:BG * (t + 1)],
                            gp.rearrange("p (j b) -> p j b", b=BG)[:, 0:KCH, :])
                        h_q = hq_new
                        c_prev = c_new
                        continue
                    # epilogue fused over groups of W chunks (slot order
                    # i,f,o,g); NFUSE=2 staggers h availability so the next
                    # step's PE overlaps the tail of the chain
                    W = KCH // nfuse
                    gp4 = gp.rearrange("p (j q b) -> p j q b", q=4, b=BG)
                    c3n = c_new.rearrange("p (j b) -> p j b", b=BG)
                    c3p = c_prev.rearrange("p (j b) -> p j b", b=BG)
                    hq3 = hq_new.rearrange("p (j b) -> p j b", b=BG)
                    for jg in range(nfuse):
                        js = slice(jg * W, (jg + 1) * W)
                        g_all = sp.tile([128, W, 4, BG], f32, name="g_all",
                                        tag=f"ga{jg}")
                        sc = (1.0 / 32.0) if fp8 else 1.0
                        nc.scalar.activation(g_all[:, :, 0:3, :],
                                             gp4[:, js, 0:3, :], AF.Sigmoid,
                                             scale=sc)
                        nc.scalar.activation(g_all[:, :, 3, :],
                                             gp4[:, js, 3, :], AF.Tanh,
                                             scale=sc)
                        cig = sp.tile([128, W, BG], f32, name="cig",
                                      tag=f"cig{jg}")
                        nc.vector.tensor_mul(cig[:], g_all[:, :, 0, :],
                                             g_all[:, :, 3, :])
                        nc.vector.tensor_mul(c3n[:, js], g_all[:, :, 1, :],
                                             c3p[:, js])
                        nc.vector.tensor_add(c3n[:, js], c3n[:, js], cig[:])
                        th = sp.tile([128, W, BG], f32, name="th", tag=f"th{jg}")
                        nc.scalar.activation(th[:], c3n[:, js], AF.Tanh)
                        nc.vector.tensor_mul(h_all[:, js, BG * t:BG * (t + 1)],
                                             g_all[:, :, 2, :], th[:])
                        if fp8:
                            nc.vector.scalar_tensor_tensor(
                                hq3[:, js], g_all[:, :, 2, :], 2.0, th[:],
                                mybir.AluOpType.mult, mybir.AluOpType.mult,
                            )
                        else:
                            nc.vector.tensor_mul(hq3[:, js], g_all[:, :, 2, :],
                                                 th[:])
                    h_q = hq_new
                    c_prev = c_new

                # batched emissions from h_all
                for n in range(GCH):
                    cols = slice(n * CW, (n + 1) * CW)
                    epb = pg.tile([L, CW], f32, name="epb", tag="gemm")
                    for k in range(KCH):
                        nc.tensor.matmul(
                            epb[:], wcls_sb[:, k, :], h_all[:, k, cols],
                            start=(k == 0), stop=(k == KCH - 1),
                        )
                    nc.vector.tensor_scalar_add(em_sb[:, cols], epb[:], bcls_sb[:])

                if phases == 'scan':
                    nc.sync.dma_start(llh_out[:], em_sb[0:1, 0:BG])
                    continue
                # ---------------- phase 3: exchange partial emissions ----------
                nc.sync.dma_start(cc_ins[rep][:], em_sb[:])
                nc.gpsimd.collective_compute(
                    "AllGather",
                    mybir.AluOpType.bypass,
                    replica_groups=[[0, 4], [1, 5], [2, 6], [3, 7]],
                    ins=[cc_ins[rep][:]],
                    outs=[cc_outs[rep][:]],
                )
                ga0 = cp.tile([L, NTOK], f32, name="ga0")
                nc.sync.dma_start(ga0[:], cc_outs[rep][0])
                ga1 = cp.tile([L, NTOK], f32, name="ga1")
                # partner slab, time-reversed within each example block
                src = cc_outs[rep][1].rearrange("p (t b) -> p t b", b=BG)
                rev = AP(src.tensor, src.offset + (T_ - 1) * BG,
                         [list(src.ap[0])] + [[-BG, T_]] + [list(src.ap[2])])
                nc.sync.dma_start(ga1.rearrange("p (t b) -> p t b", b=BG), rev)
                em_full = cp.tile([L, NTOK], f32, name="em_full")
                nc.vector.tensor_add(em_full[:], ga0[:], ga1[:])

                # ---------------- phase 4: CRF numerator ----------------
                ones_l = cp.tile([L, 1], f32, name="ones_l")
                nc.vector.memset(ones_l[:], 1.0)
                ones_r = cp.tile([1, L], f32, name="ones_r")
                nc.vector.memset(ones_r[:], 1.0)

                acc = fp.tile([L, BG], f32, name="acc", tag="acc")
                tmp_num = cp.tile([L, NTOK], f32, name="tmp_num")
                nc.vector.tensor_mul(tmp_num[:], em_full[:], ohem_sb[:])
                nc.vector.tensor_reduce(
                    acc[:], tmp_num.rearrange("p (t b) -> p b t", b=BG),
                    mybir.AxisListType.X, mybir.AluOpType.add,
                )
                # transition gather via one-hot matmul, fused multiply on eviction
                gtmp = cp.tile([L, NTOK], f32, name="gtmp")
                NG = max(1, NTOK // 512)
                for n in range(NG):
                    cols = slice(n * (NTOK // NG), (n + 1) * (NTOK // NG))
                    gpn = pg.tile([L, NTOK // NG], f32, name="gpn", tag="gemm")
                    nc.tensor.matmul(gpn[:], trans_sb[:], ohtp_sb[:, cols],
                                     start=True, stop=True)
                    nc.vector.tensor_mul(gtmp[:, cols], gpn[:], ohtt_sb[:, cols])
                acc2 = fp.tile([L, BG], f32, name="acc2", tag="acc")
                nc.vector.tensor_reduce(
                    acc2[:], gtmp.rearrange("p (t b) -> p b t", b=BG),
                    mybir.AxisListType.X, mybir.AluOpType.add,
                )
                se = fp.tile([L, 2 * BG], f32, name="se", tag="se")
                nc.vector.tensor_scalar_mul(se[:, 0:BG], ohse_sb[:, 0:BG], stv_sb[:])
                nc.vector.tensor_scalar_mul(se[:, BG:], ohse_sb[:, BG:], etv_sb[:])
                nc.vector.tensor_add(acc[:], acc[:], acc2[:])
                nc.vector.tensor_add(acc[:], acc[:], se[:, 0:BG])
                nc.vector.tensor_add(acc[:], acc[:], se[:, BG:])
                sp_ps = ps.tile([1, BG], f32, name="sp_ps", tag="small")
                nc.tensor.matmul(sp_ps[:], ones_l[:], acc[:], start=True, stop=True)
                score_sb = fp.tile([1, BG], f32, name="score_sb", tag="sc")
                nc.vector.tensor_copy(score_sb[:], sp_ps[:])

                # ---------------- phase 5: CRF denominator (linear domain) -----
                E_sb = cp.tile([L, L], f32, name="E_sb")
                nc.scalar.activation(E_sb[:], trans_sb[:], AF.Exp)
                expet = cp.tile([L, 1], f32, name="expet")
                nc.scalar.activation(expet[:], etv_sb[:], AF.Exp)
                expF = cp.tile([L, NTOK], f32, name="expF")
                nc.scalar.activation(expF[:], em_full[:], AF.Exp)

                # two interleaved chains of 4 examples; renorm side-chain
                # computed in parallel, scale applied 4 steps later (commutes
                # through the linear recursion)
                NCH = nch
                CB = BG // NCH
                aTs, bases, bcs = [], [], []
                for c2 in range(NCH):
                    off = c2 * CB * T_
                    aT = fp.tile([L, CB], f32, name=f"aT{c2}", tag=f"aT{c2}")
                    nc.scalar.activation(
                        aT[:], em_full[:, c2 * CB:(c2 + 1) * CB], AF.Exp,
                        bias=stv_sb[:])
                    aTs.append(aT)
                    base = fp.tile([1, CB], f32, name=f"base{c2}", tag=f"bs{c2}")
                    nc.vector.memset(base[:], 0.0)
                    bases.append(base)
                    bcs.append(None)

                for t in range(1, T_):
                    for c2 in range(NCH):
                        off = c2 * CB * T_ + t
                        Sp = ps.tile([L, CB], f32, name=f"Sp{c2}", tag="small")
                        nc.tensor.matmul(Sp[:], E_sb[:], aTs[c2][:],
                                         start=True, stop=True)
                        aT = fp.tile([L, CB], f32, name=f"aT{c2}", tag=f"aT{c2}")
                        nc.vector.tensor_mul(
                            aT[:], Sp[:],
                            expF[:, BG * t + c2 * CB:BG * t + (c2 + 1) * CB])
                        if bcs[c2] is not None and t % RENORM == 4:
                            nc.vector.tensor_mul(aT[:], aT[:], bcs[c2][:])
                            bcs[c2] = None
                        aTs[c2] = aT
                    if t % RENORM == 0 and t <= T_ - 5:
                        for c2 in range(NCH):
                            rp = ps.tile([1, CB], f32, name=f"rp{c2}", tag="small")
                            nc.tensor.matmul(rp[:], ones_l[:], aTs[c2][:],
                                             start=True, stop=True)
                            ls = fp.tile([1, CB], f32, name=f"ls{c2}", tag=f"ls{c2}")
                            nc.scalar.activation(ls[:], rp[:], AF.Ln)
                            base = fp.tile([1, CB], f32, name=f"base{c2}",
                                           tag=f"bs{c2}")
                            nc.vector.tensor_add(base[:], bases[c2][:], ls[:])
                            bases[c2] = base
                            rec = fp.tile([1, CB], f32, name=f"rec{c2}",
                                          tag=f"ls{c2}")
                            nc.vector.reciprocal(rec[:], rp[:])
                            bcp = ps.tile([L, CB], f32, name=f"bcp{c2}",
                                          tag="small")
                            nc.tensor.matmul(bcp[:], ones_r[:], rec[:],
                                             start=True, stop=True)
                            bc = fp.tile([L, CB], f32, name=f"bc{c2}",
                                         tag=f"bc{c2}")
                            nc.vector.tensor_copy(bc[:], bcp[:])
                            bcs[c2] = bc

                out_sb = fp.tile([1, BG], f32, name="out_sb", tag="sc")
                for c2 in range(NCH):
                    cb = slice(c2 * CB, (c2 + 1) * CB)
                    aTe = fp.tile([L, CB], f32, name=f"aTe{c2}", tag=f"aT{c2}")
                    nc.vector.tensor_scalar_mul(aTe[:], aTs[c2][:], expet[:])
                    zp = ps.tile([1, CB], f32, name=f"zp{c2}", tag="small")
                    nc.tensor.matmul(zp[:], ones_l[:], aTe[:], start=True, stop=True)
                    lz = fp.tile([1, CB], f32, name=f"lz{c2}", tag=f"ls{c2}")
                    nc.scalar.activation(lz[:], zp[:], AF.Ln)
                    nc.vector.tensor_add(out_sb[:, cb], lz[:], bases[c2][:])
                nc.vector.tensor_sub(out_sb[:], score_sb[:], out_sb[:])  # llh
                nc.sync.dma_start(llh_out[:], out_sb[:])

    nc.compile()
    return nc


# ------------------------------------------------------- time-split path ---
def build_nc2(tsplit=4, warm=16, reps=1, nch=2, phases='all', ivl=False,
              look=24, renorm=0, dbg=False, ef_bf16=True, bidi=False,
              fake_epi=False, noident=False, swi=False, xg_epi=False,
              pq_bufs=3, ps_bufs=3):
    """Time-split BiLSTM-CRF: 2*tsplit scan units (direction x window) per
    batch group; group count NG = 8 // (2*tsplit), BGc = B // NG examples
    per core. Window w>0 warms up from zero state over `warm` steps (LSTM
    state contraction); window 0's warmup inputs are exact zeros so the
    state stays exactly zero. All matmuls fp8 (DoubleRow); h state stored
    as 2*h fp8 in h_all, shared by recurrence and emissions."""
    import concourse.bass as bass
    import concourse.bacc as bacc
    import concourse.mybir as mybir
    import concourse.tile as tile
    from concourse.bass import AP

    f32 = mybir.dt.float32
    bf16 = mybir.dt.bfloat16
    f8 = mybir.dt.float8e4
    AF = mybir.ActivationFunctionType
    DR = mybir.MatmulPerfMode.DoubleRow

    NG = NCORES // (2 * tsplit)
    BGc = B // NG
    WT = T // tsplit
    TSTEPS = WT + warm
    NTOKL = BGc * TSTEPS          # local scan tokens (incl warmup)
    NTOKW = BGc * WT              # kept (emitted) tokens per window
    NTOKF = BGc * T               # full tokens per group
    CW = 256 if NTOKL % 256 == 0 else 128   # GEMM column chunk
    GCH = NTOKL // CW
    assert NTOKL % CW == 0 and NTOKW % CW == 0
    if tsplit == 4:
        groups = [list(range(8))]
    elif tsplit == 2:
        groups = [[0, 1, 2, 3], [4, 5, 6, 7]]
    else:
        raise ValueError(tsplit)

    nc = bacc.Bacc("TRN2", target_bir_lowering=False, debug=False)

    xt = nc.dram_tensor("xt", [128, ECH, NTOKL], f8, kind="ExternalInput")
    wih_shape = ([128, 2, 16, 2, 128] if swi else [128, ECH, 16, 128])
    wih = nc.dram_tensor("wih", wih_shape, f8, kind="ExternalInput")
    whh_shape = ([128, KCH // 2, 16, 2, 128] if swi
                 else [128, KCH, 16, 128])
    whh = nc.dram_tensor("whh", whh_shape, f8, kind="ExternalInput")
    LE = 32   # L padded even for DoubleRow stationary
    wcls = nc.dram_tensor("wcls", [128, KCH, LE], f8, kind="ExternalInput")
    bcls = nc.dram_tensor("bcls", [L, 1], f32, kind="ExternalInput")
    transm = nc.dram_tensor("transm", [L, L], f32, kind="ExternalInput")
    stv = nc.dram_tensor("stv", [L, 1], f32, kind="ExternalInput")
    etv = nc.dram_tensor("etv", [L, 1], f32, kind="ExternalInput")
    transmt = nc.dram_tensor("transmt", [L, L], f32, kind="ExternalInput")
    etv2 = nc.dram_tensor("etv2", [L, 1], f32, kind="ExternalInput")
    ohem = nc.dram_tensor("ohem", [L, NTOKF], bf16, kind="ExternalInput")
    numext = nc.dram_tensor("numext", [1, BGc], f32, kind="ExternalInput")
    ident = nc.dram_tensor("ident", [128, 128], f8, kind="ExternalInput")
    llh_out = nc.dram_tensor("llh_out", [1, BGc], f32, kind="ExternalOutput")
    if dbg:
        demf = nc.dram_tensor("demf", [L, BGc * T], f32, kind="ExternalOutput")
        dsc = nc.dram_tensor("dsc", [1, BGc], f32, kind="ExternalOutput")
        dxg = nc.dram_tensor("dxg", [128, 16, BGc * (T // tsplit + warm)], f8,
                             kind="ExternalOutput")

    cc_ins = [nc.dram_tensor(f"cc_in{r}", [L, NTOKW], f32)
              for r in range(reps)]
    cc_space = "Shared" if tsplit == 4 else "Local"
    cc_outs = [nc.dram_tensor(f"cc_out{r}", [2 * tsplit, L, NTOKW], f32,
                              addr_space=cc_space)
               for r in range(reps)]

    with tile.TileContext(nc) as tc:
        with tc.tile_pool(name="const", bufs=1) as cp, \
             tc.tile_pool(name="state", bufs=3) as sp, \
             tc.tile_pool(name="crf", bufs=3) as fp, \
             tc.tile_pool(name="stage", bufs=2) as bp, \
             tc.tile_pool(name="pgemm", bufs=2, space="PSUM") as pg, \
             tc.tile_pool(name="pgates", bufs=pq_bufs, space="PSUM") as pq, \
             tc.tile_pool(name="psmall", bufs=ps_bufs, space="PSUM") as ps:

            # ---------------- loads ----------------
            xt_sb = cp.tile([128, ECH, NTOKL], f8, name="xt_sb")
            nc.sync.dma_start(xt_sb[:], xt[:])
            wih_sb = cp.tile(wih_shape, f8, name="wih_sb")
            nc.sync.dma_start(wih_sb[:], wih[:])
            whh_sb = cp.tile(whh_shape, f8, name="whh_sb")
            nc.sync.dma_start(whh_sb[:], whh[:])
            wcls_sb = cp.tile([128, KCH, LE], f8, name="wcls_sb")
            nc.sync.dma_start(wcls_sb[:], wcls[:])
            bcls_sb = cp.tile([L, 1], f32, name="bcls_sb")
            nc.sync.dma_start(bcls_sb[:], bcls[:])
            trans_sb = cp.tile([L, L], f32, name="trans_sb")
            nc.sync.dma_start(trans_sb[:], transm[:])
            stv_sb = cp.tile([L, 1], f32, name="stv_sb")
            nc.sync.dma_start(stv_sb[:], stv[:])
            etv_sb = cp.tile([L, 1], f32, name="etv_sb")
            nc.sync.dma_start(etv_sb[:], etv[:])
            transmt_sb = cp.tile([L, L], f32, name="transmt_sb")
            nc.sync.dma_start(transmt_sb[:], transmt[:])
            etv2_sb = cp.tile([L, 1], f32, name="etv2_sb")
            nc.sync.dma_start(etv2_sb[:], etv2[:])
            ohem_sb = cp.tile([L, NTOKF], bf16, name="ohem_sb")
            nc.sync.dma_start(ohem_sb[:], ohem[:])
            numext_sb = cp.tile([1, BGc], f32, name="numext_sb")
            nc.sync.dma_start(numext_sb[:], numext[:])
            ident_sb = cp.tile([128, 128], f8, name="ident_sb")
            nc.sync.dma_start(ident_sb[:], ident[:])

            xg_sb = cp.tile([128, 16, NTOKL], f8, name="xg_sb")
            em_sb = cp.tile([L, NTOKW], f32, name="em_sb")
            em_full = cp.tile([L, NTOKF], f32, name="em_full")
            expF = cp.tile([L, NTOKF], bf16 if ef_bf16 else f32,
                           name="expF")
            ones_l = cp.tile([L, 1], f32, name="ones_l")
            nc.vector.memset(ones_l[:], 1.0)
            ones_r = cp.tile([1, L], f32, name="ones_r")
            nc.vector.memset(ones_r[:], 1.0)

            for rep in range(reps):
                # ---------------- phase 1: input GEMM (fp8 DR) ----------
                gem_units = [(n, s) for n in range(GCH) for s in range(16)]
                gu_state = [0]

                DRSWI2 = mybir.MatmulPerfMode.DoubleRowSwInterleave

                def emit_gemm_unit():
                    n, s = gem_units[gu_state[0]]
                    gu_state[0] += 1
                    cols = slice(n * CW, (n + 1) * CW)
                    gp = pg.tile([128, CW], f32, name="gp", tag="gemm")
                    nc.tensor.matmul(
                        gp[:],
                        wih_sb[:, 0, s, :, :] if swi
                        else wih_sb[:, 0:2, s, :],
                        xt_sb[:, 0:2, cols],
                        start=True, stop=False,
                        perf_mode=DRSWI2 if swi else DR,
                    )
                    nc.tensor.matmul(
                        gp[:],
                        wih_sb[:, 1, s, 0, :] if swi
                        else wih_sb[:, 2, s, :],
                        xt_sb[:, 2, cols],
                        start=False, stop=True,
                    )
                    if s % 2 == 1:
                        nc.scalar.copy(xg_sb[:, s, cols], gp[:])
                    else:
                        nc.vector.tensor_copy(xg_sb[:, s, cols], gp[:])

                SPC = CW // BGc        # scan steps covered per GEMM chunk

                def gemm_until(step):
                    tgt = min(len(gem_units),
                              ((step + SPC - 1) // SPC + 1) * 16)
                    while gu_state[0] < tgt:
                        emit_gemm_unit()

                def emit_em_chunk(m):
                    ecols = slice((warm + 1) * BGc + m * CW,
                                  (warm + 1) * BGc + (m + 1) * CW)
                    cols = slice(m * CW, (m + 1) * CW)
                    epb = pg.tile([LE, CW], f32, name="epb", tag="gemm")
                    for kp in range(2):
                        nc.tensor.matmul(
                            epb[:], wcls_sb[:, 2 * kp:2 * kp + 2, :],
                            h_all[:, 2 * kp:2 * kp + 2, ecols],
                            start=(kp == 0), stop=(kp == 1), perf_mode=DR,
                        )
                    nc.scalar.activation(em_sb[:, cols], epb[0:L, :],
                                         AF.Identity, bias=bcls_sb[:],
                                         scale=1.0 / 64.0)

                if not ivl:
                    gemm_until(10 ** 9)

                # ---------------- phase 2: LSTM scan --------------------
                # h_all[:, :, t*BGc:(t+1)*BGc] holds 2*h_{t-1} in fp8;
                # slice 0 is the zero initial state.
                h_all = cp.tile([128, KCH, (TSTEPS + 1) * BGc], f8,
                                name="h_all")
                nc.vector.memset(h_all[:, :, 0:BGc], 0.0)
                c_prev = sp.tile([128, KCH * BGc], f32, name="c_prev",
                                 tag="c")
                nc.vector.memset(c_prev[:], 0.0)
                if ivl:
                    gemm_until(look)   # cover the scan's first `look` steps

                if xg_epi:
                    noident = True
                for t in range(TSTEPS):
                    gp = pq.tile([128, 16 * BGc], f32, name="gp_scan",
                                 tag="g")
                    gpv = gp.rearrange("p (s b) -> p s b", b=BGc)
                    if not noident:
                        nc.tensor.matmul(
                            gp[:], ident_sb[:],
                            xg_sb[:, :, BGc * t:BGc * (t + 1)],
                            start=True, stop=False, skip_group_check=True,
                        )
                    hprev = h_all[:, :, BGc * t:BGc * (t + 1)]
                    DRSWI = mybir.MatmulPerfMode.DoubleRowSwInterleave
                    for kp in range(2):
                        for s in range(16):
                            lw = (whh_sb[:, kp, s, :, :] if swi
                                  else whh_sb[:, 2 * kp:2 * kp + 2, s, :])
                            nc.tensor.matmul(
                                gpv[:, s, :], lw,
                                hprev[:, 2 * kp:2 * kp + 2, :],
                                start=(noident and kp == 0), stop=(kp == 1),
                                perf_mode=(DRSWI if swi else DR),
                                skip_group_check=True,
                            )
                    c_new = sp.tile([128, KCH * BGc], f32, name="c_new",
                                    tag="c")
                    if fake_epi:
                        nc.vector.tensor_copy(
                            h_all.rearrange("p j n -> p (j n)")[
                                :, 0:KCH * BGc],
                            gp[:, 0:KCH * BGc])
                        c_prev = c_new
                        if ivl:
                            gemm_until(t + 1 + look)
                        continue
                    gp4 = gp.rearrange("p (j q b) -> p j q b", q=4, b=BGc)
                    c3n = c_new.rearrange("p (j b) -> p j b", b=BGc)
                    c3p = c_prev.rearrange("p (j b) -> p j b", b=BGc)
                    hnxt = h_all[:, :, BGc * (t + 1):BGc * (t + 2)]
                    for jg in range(2):
                        js = slice(jg * 2, jg * 2 + 2)
                        g_all = sp.tile([128, 2, 4, BGc], f32, name="g_all",
                                        tag=f"ga{jg}")
                        if xg_epi:
                            # gates = 2*(16 xg) + psum(32 Wh) = 32*(xg+Wh)
                            gsum = sp.tile([128, 2, 4, BGc], f32,
                                           name="gsum", tag=f"gs{jg}")
                            nc.vector.scalar_tensor_tensor(
                                gsum[:],
                                xg_sb[:, 8 * jg:8 * jg + 8,
                                      BGc * t:BGc * (t + 1)].rearrange(
                                    "p (j q) b -> p j q b", q=4),
                                2.0,
                                gp4[:, js, :, :],
                                mybir.AluOpType.mult, mybir.AluOpType.add,
                            )
                            gsrc = gsum
                        else:
                            gsrc = gp4[:, js, :, :]
                        nc.scalar.activation(g_all[:, :, 0:3, :],
                                             gsrc[:, :, 0:3, :], AF.Sigmoid,
                                             scale=1.0 / 32.0)
                        nc.scalar.activation(g_all[:, :, 3, :],
                                             gsrc[:, :, 3, :], AF.Tanh,
                                             scale=1.0 / 32.0)
                        cig = sp.tile([128, 2, BGc], f32, name="cig",
                                      tag=f"cig{jg}")
                        nc.vector.tensor_mul(cig[:], g_all[:, :, 0, :],
                                             g_all[:, :, 3, :])
                        nc.vector.tensor_mul(c3n[:, js], g_all[:, :, 1, :],
                                             c3p[:, js])
                        nc.vector.tensor_add(c3n[:, js], c3n[:, js], cig[:])
                        th = sp.tile([128, 2, BGc], f32, name="th",
                                     tag=f"th{jg}")
                        nc.scalar.activation(th[:], c3n[:, js], AF.Tanh)
                        nc.vector.scalar_tensor_tensor(
                            hnxt[:, js], g_all[:, :, 2, :], 2.0, th[:],
                            mybir.AluOpType.mult, mybir.AluOpType.mult,
                        )
                    c_prev = c_new
                    if ivl:
                        # fill the PE stall behind this step's epilogue
                        gemm_until(t + 1 + look)
                        if t >= warm and (t - warm) % SPC == SPC - 1:
                            m = (t - warm) // SPC
                            if m < NTOKW // CW - 1:
                                emit_em_chunk(m)

                # emissions over kept region (fp8 DR, evict w/ scale+bias)
                em_done = (NTOKW // CW - 1) if ivl else 0
                for m in range(em_done, NTOKW // CW):
                    emit_em_chunk(m)

                if phases == 'scan':
                    nc.sync.dma_start(llh_out[:], em_sb[0:1, 0:BGc])
                    continue

                # ---------------- phase 3: exchange + assemble ----------
                nc.sync.dma_start(cc_ins[rep][:], em_sb[:])
                nc.gpsimd.collective_compute(
                    "AllGather",
                    mybir.AluOpType.bypass,
                    replica_groups=groups,
                    ins=[cc_ins[rep][:]],
                    outs=[cc_outs[rep][:]],
                )
                for w in range(tsplit):
                    nc.sync.dma_start(
                        em_full[:, w * NTOKW:(w + 1) * NTOKW],
                        cc_outs[rep][w])
                worder = ([0, tsplit - 1] + list(range(1, tsplit - 1))
                          if tsplit > 1 else [0])
                for w in worder:
                    w2 = tsplit - 1 - w
                    src = cc_outs[rep][tsplit + w2].rearrange(
                        "p (t b) -> p t b", b=BGc)
                    rev = AP(src.tensor, src.offset + (WT - 1) * BGc,
                             [list(src.ap[0])] + [[-BGc, WT]]
                             + [list(src.ap[2])])
                    bs = bp.tile([L, NTOKW], f32, name="bs", tag="bs")
                    nc.sync.dma_start(
                        bs.rearrange("p (t b) -> p t b", b=BGc), rev)
                    nc.vector.tensor_add(
                        em_full[:, w * NTOKW:(w + 1) * NTOKW],
                        em_full[:, w * NTOKW:(w + 1) * NTOKW], bs[:])

                if dbg and rep == 0:
                    nc.sync.dma_start(demf[:], em_full[:])
                # ---------------- phase 4: numerator --------------------
                acc = fp.tile([L, BGc], f32, name="acc", tag="acc")
                for chn in range(NTOKF // NTOKW):
                    cs = slice(chn * NTOKW, (chn + 1) * NTOKW)
                    tm = bp.tile([L, NTOKW], f32, name="tm", tag="bs")
                    nc.gpsimd.tensor_mul(tm[:], em_full[:, cs],
                                         ohem_sb[:, cs])
                    rc = fp.tile([L, BGc], f32, name="rc", tag="rc")
                    nc.vector.tensor_reduce(
                        rc[:], tm.rearrange("p (t b) -> p b t", b=BGc),
                        mybir.AxisListType.X, mybir.AluOpType.add,
                    )
                    if chn == 0:
                        nc.gpsimd.tensor_copy(acc[:], rc[:])
                    else:
                        nc.gpsimd.tensor_add(acc[:], acc[:], rc[:])
                sp_ps = ps.tile([1, BGc], f32, name="sp_ps", tag="small")
                nc.tensor.matmul(sp_ps[:], ones_l[:], acc[:], start=True,
                                 stop=True)
                score_sb = fp.tile([1, BGc], f32, name="score_sb", tag="sc")
                nc.vector.tensor_add(score_sb[:], sp_ps[:], numext_sb[:])
                if dbg and rep == 0:
                    nc.sync.dma_start(dsc[:], score_sb[:])
                    nc.sync.dma_start(dxg[:], xg_sb[:])

                # ---------------- phase 5: denominator ------------------
                E_sb = cp.tile([L, L], f32, name="E_sb")
                nc.scalar.activation(E_sb[:], trans_sb[:], AF.Exp)
                expet = cp.tile([L, 1], f32, name="expet")
                nc.scalar.activation(expet[:], etv_sb[:], AF.Exp)
                ebias_sb = cp.tile([L, 1], f32, name="ebias_sb")
                nc.vector.memset(ebias_sb[:],
                                 -float(np.log(L)) if renorm == 0 else 0.0)
                CHN = NTOKF // NTOKW
                chn_order = ([0, CHN - 1] + list(range(1, CHN - 1))
                             if CHN > 1 else [0])
                for chn in chn_order:
                    cs = slice(chn * NTOKW, (chn + 1) * NTOKW)
                    nc.scalar.activation(expF[:, cs], em_full[:, cs], AF.Exp,
                                         bias=ebias_sb[:])

                CB = BGc // nch
                if bidi:
                    ET_sb = cp.tile([L, L], f32, name="ET_sb")
                    nc.scalar.activation(ET_sb[:], transmt_sb[:], AF.Exp)
                    aTs, bTs = [], []
                    for c2 in range(nch):
                        aT = fp.tile([L, CB], f32, name=f"aT{c2}",
                                     tag=f"aT{c2}")
                        nc.scalar.activation(
                            aT[:], em_full[:, c2 * CB:(c2 + 1) * CB], AF.Exp,
                            bias=stv_sb[:])
                        aTs.append(aT)
                        bT = fp.tile([L, CB], f32, name=f"bT{c2}",
                                     tag=f"bT{c2}")
                        nc.scalar.activation(
                            bT[:],
                            em_full[:, (T - 1) * BGc + c2 * CB:
                                    (T - 1) * BGc + (c2 + 1) * CB],
                            AF.Exp, bias=etv2_sb[:])
                        bTs.append(bT)
                    for u in range(1, T // 2):
                        tb = T - 1 - u
                        for c2 in range(nch):
                            Sp = ps.tile([L, CB], f32, name=f"Sp{c2}",
                                         tag="small")
                            nc.tensor.matmul(Sp[:], E_sb[:], aTs[c2][:],
                                             start=True, stop=True)
                            aT = fp.tile([L, CB], f32, name=f"aT{c2}",
                                         tag=f"aT{c2}")
                            nc.vector.tensor_mul(
                                aT[:], Sp[:],
                                expF[:, BGc * u + c2 * CB:
                                     BGc * u + (c2 + 1) * CB])
                            aTs[c2] = aT
                        for c2 in range(nch):
                            Spb = pg.tile([L, CB], f32, name=f"Spb{c2}",
                                          tag="gemm")
                            nc.tensor.matmul(Spb[:], ET_sb[:], bTs[c2][:],
                                             start=True, stop=True)
                            bT = fp.tile([L, CB], f32, name=f"bT{c2}",
                                         tag=f"bT{c2}")
                            nc.vector.tensor_mul(
                                bT[:], Spb[:],
                                expF[:, BGc * tb + c2 * CB:
                                     BGc * tb + (c2 + 1) * CB])
                            bTs[c2] = bT
                    out_sb = fp.tile([1, BGc], f32, name="out_sb", tag="sc")
                    for c2 in range(nch):
                        cb = slice(c2 * CB, (c2 + 1) * CB)
                        vp = ps.tile([L, CB], f32, name=f"vp{c2}",
                                     tag="small")
                        nc.tensor.matmul(vp[:], ET_sb[:], bTs[c2][:],
                                         start=True, stop=True)
                        t2 = fp.tile([L, CB], f32, name=f"t2{c2}",
                                     tag=f"aT{c2}")
                        nc.vector.tensor_mul(t2[:], aTs[c2][:], vp[:])
                        zp = ps.tile([1, CB], f32, name=f"zp{c2}",
                                     tag="small")
                        nc.tensor.matmul(zp[:], ones_l[:], t2[:],
                                         start=True, stop=True)
                        nc.scalar.activation(out_sb[0:1, cb], zp[:], AF.Ln)
                    nc.vector.tensor_sub(out_sb[:], score_sb[:], out_sb[:])
                    nc.sync.dma_start(llh_out[:], out_sb[:])
                    continue
                aTs, bases, bcs = [], [], []
                for c2 in range(nch):
                    aT = fp.tile([L, CB], f32, name=f"aT{c2}", tag=f"aT{c2}")
                    nc.scalar.activation(
                        aT[:], em_full[:, c2 * CB:(c2 + 1) * CB], AF.Exp,
                        bias=stv_sb[:])
                    aTs.append(aT)
                    base = fp.tile([1, CB], f32, name=f"base{c2}",
                                   tag=f"bs{c2}")
                    nc.vector.memset(base[:], 0.0)
                    bases.append(base)
                    bcs.append(None)

                for t in range(1, T):
                    for c2 in range(nch):
                        Sp = ps.tile([L, CB], f32, name=f"Sp{c2}",
                                     tag="small")
                        nc.tensor.matmul(Sp[:], E_sb[:], aTs[c2][:],
                                         start=True, stop=True)
                        aT = fp.tile([L, CB], f32, name=f"aT{c2}",
                                     tag=f"aT{c2}")
                        nc.vector.tensor_mul(
                            aT[:], Sp[:],
                            expF[:, BGc * t + c2 * CB:BGc * t + (c2 + 1) * CB])
                        if bcs[c2] is not None and t % renorm == 4:
                            nc.vector.tensor_mul(aT[:], aT[:], bcs[c2][:])
                            bcs[c2] = None
                        aTs[c2] = aT
                    if renorm and t % renorm == 0 and t <= T - 5:
                        for c2 in range(nch):
                            rp = ps.tile([1, CB], f32, name=f"rp{c2}",
                                         tag="small")
                            nc.tensor.matmul(rp[:], ones_l[:], aTs[c2][:],
                                             start=True, stop=True)
                            ls = fp.tile([1, CB], f32, name=f"ls{c2}",
                                         tag=f"ls{c2}")
                            nc.scalar.activation(ls[:], rp[:], AF.Ln)
                            base = fp.tile([1, CB], f32, name=f"base{c2}",
                                           tag=f"bs{c2}")
                            nc.vector.tensor_add(base[:], bases[c2][:], ls[:])
                            bases[c2] = base
                            rec = fp.tile([1, CB], f32, name=f"rec{c2}",
                                          tag=f"ls{c2}")
                            # exp(-ln(rp)): DVE reciprocal loses accuracy
                            # for rp ~ 1e20 (renorm=16 mass growth)
                            nc.scalar.activation(rec[:], ls[:], AF.Exp,
                                                 scale=-1.0)
                            bcp = ps.tile([L, CB], f32, name=f"bcp{c2}",
                                          tag="small")
                            nc.tensor.matmul(bcp[:], ones_r[:], rec[:],
                                             start=True, stop=True)
                            bc = fp.tile([L, CB], f32, name=f"bc{c2}",
                                         tag=f"bc{c2}")
                            nc.vector.tensor_copy(bc[:], bcp[:])
                            bcs[c2] = bc

                out_sb = fp.tile([1, BGc], f32, name="out_sb", tag="sc")
                for c2 in range(nch):
                    cb = slice(c2 * CB, (c2 + 1) * CB)
                    aTe = fp.tile([L, CB], f32, name=f"aTe{c2}",
                                  tag=f"aT{c2}")
                    nc.vector.tensor_scalar_mul(aTe[:], aTs[c2][:], expet[:])
                    zp = ps.tile([1, CB], f32, name=f"zp{c2}", tag="small")
                    nc.tensor.matmul(zp[:], ones_l[:], aTe[:], start=True,
                                     stop=True)
                    lz = fp.tile([1, CB], f32, name=f"lz{c2}", tag=f"ls{c2}")
                    nc.scalar.activation(lz[:], zp[:], AF.Ln)
                    nc.vector.tensor_add(out_sb[:, cb], lz[:], bases[c2][:])
                nc.vector.tensor_sub(out_sb[:], score_sb[:], out_sb[:])
                nc.sync.dma_start(llh_out[:], out_sb[:])

    nc.compile()
    return nc


def _pack_core2(x_win, bias_col, w_ih, w_hh, b_ih, b_hh, w_cls_half,
                bcls_val, trans, st, et, labels_g, mask_g, tsplit, warm,
                swi=False):
    """x_win: [BGc, TSTEPS, E] fp32 scan-ordered window (warmup included,
    zeros for window 0's warmup); bias_col: [BGc, TSTEPS] (0 during window
    0's warmup, else 1)."""
    BGc, TSTEPS, _ = x_win.shape
    NTOKL = BGc * TSTEPS
    T_ = T
    xt = np.zeros([EPAD, NTOKL], np.float32)
    xt[:E] = x_win.transpose(1, 0, 2).reshape(TSTEPS * BGc, E).T
    xt[E] = bias_col.T.reshape(-1)
    xt_dev = np.ascontiguousarray(
        xt.reshape(ECH, 128, NTOKL).transpose(1, 0, 2)).astype(f8l)

    w_ih_aug = np.zeros([4 * H, EPAD], np.float32)
    w_ih_aug[:, :E] = w_ih
    w_ih_aug[:, E] = b_ih + b_hh
    wih_dev = np.zeros([128, ECH, 16, 128], np.float32)
    whh_dev = np.zeros([128, KCH, 16, 128], np.float32)
    for s in range(16):
        r = _slot_rows(s)
        for k in range(ECH):
            wih_dev[:, k, s, :] = w_ih_aug[r:r + 128, k * 128:(k + 1) * 128].T
        for k in range(KCH):
            whh_dev[:, k, s, :] = w_hh[r:r + 128, k * 128:(k + 1) * 128].T
    wcls_dev = np.zeros([128, KCH, 32], np.float32)
    for k in range(KCH):
        wcls_dev[:, k, :L] = w_cls_half[:, k * 128:(k + 1) * 128].T

    # numerator: emission one-hots on device; trans/start/end on host
    m = mask_g.astype(np.float64)
    NTOKF = BGc * T_
    ohem = np.zeros([L, NTOKF], np.float32)
    numext = np.zeros([BGc], np.float64)
    for b in range(BGc):
        lab = labels_g[b]
        numext[b] += st[lab[0]] + et[lab[int(m[b].sum()) - 1]]
        for t in range(T_):
            w8 = 1.0 if t == 0 else m[b, t]
            ohem[lab[t], t * BGc + b] += w8
            if t >= 1 and m[b, t] > 0:
                numext[b] += trans[lab[t - 1], lab[t]]

    def _swi_pack(wd, kch):
        out = np.zeros([128, kch // 2, 16, 2, 128], np.float32)
        for kp in range(kch // 2):
            A = wd[:, 2 * kp, :, ::-1]
            Bm = wd[:, 2 * kp + 1, :, ::-1]
            out[:, kp, :, :, :] = np.stack(
